# revision 13
# baseline (speedup 1.0000x reference)
"""AdaAttNStar fused kernel for 8 TRN2 NeuronCores.

Algebraic collapse: the reference builds A = Q^T K ([B, N, N]) explicitly, but
  M  = V A^T        = (V K^T) Q        ([B,C,C] Gram @ [B,C,N])
  S2 = V^2 A^T - M^2 = (V^2 K^T) Q - M^2
so the whole computation reduces to channel Grams ([B,3,3]), per-channel
normalization statistics, and one elementwise epilogue over [B,C,N].

Distribution: stats/Grams are global reductions, and on-chip collectives have a
multi-microsecond latency floor, so every core redundantly computes the (cheap)
reduction phase from the full inputs and epilogues only its 1/8 slice of N.
Per-core output slices are concatenated on the host.

Layouts: big tensors live in SBUF as [128, (b, c, f=72)] with spatial index
s = p*72 + f; reductions over s happen free-dim-first (DVE per partition) then
across partitions with a ones-column matmul on the PE.
"""

import os

import numpy as np

import concourse.bass as bass
import concourse.bacc as bacc
import concourse.tile as tile
from concourse import mybir
from concourse.bass_utils import run_bass_kernel_spmd

B, C, N = 2, 3, 9216
P, F = 128, 72            # N = P * F
NCORES = 8
NS, FS = N // NCORES, F // NCORES     # 1152, 9
MTOT = B * N              # 18432 elements per channel for the norm stats
EPS = 1e-12
f32 = mybir.dt.float32
Alu = mybir.AluOpType
Act = mybir.ActivationFunctionType

FULL_INPUTS = ["F_c", "F_s", "F_c_previous", "F_s_previous"]
STAGE = int(os.environ.get("KERNEL_STAGE", "3"))

# sc scratch column map (scalar phase, everything on partition 0)
SC_R = 0        # Gram sums, order (w,b,i,j)                [36]
SC_SUM = 36     # sum_s X per (t,b,c), t=(fc,fcp,fsp,fs)    [24]
SC_SS = 60      # sum_s X^2 per (t,b,c)                     [24]
SC_UP = 84      # b-pooled sums, t in {fc,fcp,fsp}          [9]
SC_SSP = 93     # b-pooled sumsq                            [9]
SC_Q = 102      # UP^2                                      [9]
SC_VARM = 111   # SSP - Q/MTOT  (= var*(MTOT-1))            [9]
SC_STD = 120    # sqrt(VARM/(MTOT-1))                       [9]
SC_SE = 129     # STD + eps                                 [9]
SC_A = 138      # 1/(STD+eps): ac=138, aq=141, ak=144       [9]
SC_GRID = 147   # sv_w[b,i] * UP[fsp, j]                    [36]
SC_RC = 183     # R - GRID/MTOT                             [36]
SC_AKQ = 219    # ak*aq                                     [3]
SC_H = 222      # RC*akq_j     <- broadcast region start    [36]
SC_H0 = 258     # sum_j H*mq_j                              [12]
SC_AC = 270     # ac copy                                   [3]
SC_MCAC = 273   # mc*ac = UP[fc]*ac/MTOT                    [3]
SC_HM = 276     # scratch H*UP[fcp,j]/MTOT                  [36]
SC_END = 312


def _body(tc, dr, out_sl):
    nc = tc.nc
    with (
        tc.tile_pool(name="main", bufs=1) as pool,
        tc.tile_pool(name="scr", bufs=4) as scr,
        tc.tile_pool(name="ep", bufs=2) as ep,
        tc.tile_pool(name="ps", bufs=1, space="PSUM") as pp,
    ):
        # ---- input DMAs -------------------------------------------------
        t = {}
        for name, key in [("F_s", "fs"), ("F_s_previous", "fsp"),
                          ("F_c_previous", "fcp"), ("F_c", "fc")]:
            tl = pool.tile([P, B * C * F], f32, tag=key)
            nc.sync.dma_start(
                tl[:].rearrange("p (b c f) -> p b c f", b=B, c=C),
                dr[name].ap().rearrange("b c (p f) -> p b c f", p=P))
            t[key] = tl
        fcp_sl = pool.tile([P, B * C * FS], f32, tag="fcpsl")
        nc.gpsimd.dma_start(
            fcp_sl[:].rearrange("p (b c f) -> p b c f", b=B, c=C),
            dr["F_cp_sl"].ap().rearrange("b c (p f) -> p b c f", p=P))
        fc_sl = pool.tile([P, B * C * FS], f32, tag="fcsl")
        nc.gpsimd.dma_start(
            fc_sl[:].rearrange("p (b c f) -> p b c f", b=B, c=C),
            dr["F_c_sl"].ap().rearrange("b c (p f) -> p b c f", p=P))

        def v4(tl, f=F):   # [128, (b c f)] -> [128, b, c, f]
            return tl[:].rearrange("p (b c f) -> p b c f", b=B, c=C, f=f)

        V = nc.vector
        if STAGE == 0:
            out_t = ep.tile([P, B * C * FS], f32, tag="outt")
            V.tensor_copy(out_t[:], fc_sl[:])
            nc.sync.dma_start(
                out_sl.ap().rearrange("b c (p f) -> p b c f", p=P),
                out_t[:].rearrange("p (b c f) -> p b c f", b=B, c=C))
            return

        # ---- squares (ACT) ---------------------------------------------
        v2 = pool.tile([P, B * C * F], f32, tag="v2")
        nc.scalar.activation(v2[:], t["fs"][:], Act.Square)
        sq = {"fs": v2}
        for key in ["fc", "fcp", "fsp"]:
            s = pool.tile([P, B * C * F], f32, tag=f"sq{key}")
            nc.scalar.activation(s[:], t[key][:], Act.Square)
            sq[key] = s

        ones_col = pool.tile([P, 1], f32, tag="onesc")
        nc.gpsimd.memset(ones_col[:], 1.0)
        ones_row = pool.tile([1, P], f32, tag="onesr")
        nc.gpsimd.memset(ones_row[:], 1.0)

        if STAGE == 11:
            out_t = ep.tile([P, B * C * FS], f32, tag="outt")
            V.tensor_copy(out_t[:], v2[:, 0:54])
            nc.sync.dma_start(
                out_sl.ap().rearrange("b c (p f) -> p b c f", p=P),
                out_t[:].rearrange("p (b c f) -> p b c f", b=B, c=C))
            return

        # ---- per-partition partial reductions (DVE) --------------------
        # acc cols: 0:36 Gram (w,b,i,j); 36:60 sums (t,b,c); 60:84 sumsq
        acc = pool.tile([P, 84], f32, tag="acc")
        fsv, v2v, fspv = v4(t["fs"]), v4(v2), v4(t["fsp"])
        for w, src in enumerate([] if STAGE == 12 else [fsv, v2v]):
            for b in range(B):
                for i in range(C):
                    for j in range(C):
                        q = ((w * B + b) * C + i) * C + j
                        wscr = scr.tile([P, F], f32, tag="wscr")
                        nc.vector.scalar_tensor_tensor(
                            out=wscr[:],
                            in0=src[:, b, i, :], scalar=1.0,
                            in1=fspv[:, b, j, :],
                            op0=Alu.mult, op1=Alu.mult,
                            accum_out=acc[:, q:q + 1])
        for ti, key in enumerate(["fc", "fcp", "fsp", "fs"]):
            nc.vector.reduce_sum(
                acc[:, 36 + ti * 6:42 + ti * 6].rearrange(
                    "p (b c) -> p b c", b=B),
                v4(t[key]), axis=mybir.AxisListType.X)
            nc.vector.reduce_sum(
                acc[:, 60 + ti * 6:66 + ti * 6].rearrange(
                    "p (b c) -> p b c", b=B),
                v4(sq[key]), axis=mybir.AxisListType.X)

        if STAGE in (1, 12):
            out_t = ep.tile([P, B * C * FS], f32, tag="outt")
            V.tensor_copy(out_t[:, 0:48] if STAGE == 12 else out_t[:],
                          acc[:, 36:84] if STAGE == 12 else acc[:, 0:54])
            if STAGE == 12:
                V.tensor_copy(out_t[:, 48:54], acc[:, 36:42])
            nc.sync.dma_start(
                out_sl.ap().rearrange("b c (p f) -> p b c f", p=P),
                out_t[:].rearrange("p (b c f) -> p b c f", b=B, c=C))
            return

        # ---- cross-partition reduction on the PE -----------------------
        p1 = pp.tile([1, 96], f32, tag="p1")
        nc.tensor.matmul(p1[:, 0:84], ones_col[:], acc[:, 0:84],
                         start=True, stop=True)

        # ---- scalar phase on [1, *] ------------------------------------
        sc = pool.tile([1, 512], f32, tag="sc")

        def scs(base, n):
            return sc[:, base:base + n]

        V.tensor_copy(scs(0, 84), p1[:, 0:84])
        sum_t = scs(SC_SUM, 24).rearrange("o (t b c) -> o t b c", t=4, b=B)
        ss_t = scs(SC_SS, 24).rearrange("o (t b c) -> o t b c", t=4, b=B)
        V.tensor_add(scs(SC_UP, 9), sum_t[:, 0:3, 0, :], sum_t[:, 0:3, 1, :])
        V.tensor_add(scs(SC_SSP, 9), ss_t[:, 0:3, 0, :], ss_t[:, 0:3, 1, :])
        V.tensor_mul(scs(SC_Q, 9), scs(SC_UP, 9), scs(SC_UP, 9))
        V.scalar_tensor_tensor(
            out=scs(SC_VARM, 9), in0=scs(SC_Q, 9), scalar=-1.0 / MTOT,
            in1=scs(SC_SSP, 9), op0=Alu.mult, op1=Alu.add)
        nc.scalar.activation(scs(SC_STD, 9), scs(SC_VARM, 9), Act.Sqrt,
                             scale=1.0 / (MTOT - 1))
        V.tensor_scalar_add(scs(SC_SE, 9), scs(SC_STD, 9), EPS)
        V.reciprocal(scs(SC_A, 9), scs(SC_SE, 9))

        # grid[w,b,i,j] = sv_w[b,i] * UP[fsp, j];  sv_0 = SUM[fs], sv_1 = SS[fs]
        up_fsp = scs(SC_UP + 6, 3)       # UP[t=2 (fsp)]
        for w, base in enumerate([SC_SUM + 18, SC_SS + 18]):
            sv_ap = scs(base, 6)         # [o, (b i)]
            V.tensor_mul(
                scs(SC_GRID + w * 18, 18).rearrange("o (g j) -> o g j", j=C),
                sv_ap.unsqueeze(2).broadcast_to((1, 6, 3)),
                up_fsp.unsqueeze(1).broadcast_to((1, 6, 3)))
        V.scalar_tensor_tensor(
            out=scs(SC_RC, 36), in0=scs(SC_GRID, 36), scalar=-1.0 / MTOT,
            in1=scs(SC_R, 36), op0=Alu.mult, op1=Alu.add)
        V.tensor_mul(scs(SC_AKQ, 3), scs(SC_A + 6, 3), scs(SC_A + 3, 3))
        V.tensor_mul(
            scs(SC_H, 36).rearrange("o (g j) -> o g j", j=C),
            scs(SC_RC, 36).rearrange("o (g j) -> o g j", j=C),
            scs(SC_AKQ, 3).unsqueeze(1).broadcast_to((1, 12, 3)))
        V.scalar_tensor_tensor(
            out=scs(SC_HM, 36).rearrange("o (g j) -> o g j", j=C),
            in0=scs(SC_H, 36).rearrange("o (g j) -> o g j", j=C),
            scalar=1.0 / MTOT,
            in1=scs(SC_UP + 3, 3).unsqueeze(1).broadcast_to((1, 12, 3)),
            op0=Alu.mult, op1=Alu.mult)
        V.reduce_sum(scs(SC_H0, 12),
                     scs(SC_HM, 36).rearrange("o (g j) -> o g j", j=C),
                     axis=mybir.AxisListType.X)
        V.tensor_copy(scs(SC_AC, 3), scs(SC_A, 3))
        V.scalar_tensor_tensor(
            out=scs(SC_MCAC, 3), in0=scs(SC_UP, 3), scalar=1.0 / MTOT,
            in1=scs(SC_A, 3), op0=Alu.mult, op1=Alu.mult)

        if STAGE == 2:
            out_t = ep.tile([P, B * C * FS], f32, tag="outt")
            nc.gpsimd.memset(out_t[:], 0.0)
            V.tensor_copy(out_t[0:1, :], sc[:, 0:54])
            nc.sync.dma_start(
                out_sl.ap().rearrange("b c (p f) -> p b c f", p=P),
                out_t[:].rearrange("p (b c f) -> p b c f", b=B, c=C))
            return

        # ---- broadcast the 54 epilogue scalars to all partitions -------
        pbc = pp.tile([P, 64], f32, tag="pbc")
        nc.tensor.matmul(pbc[:, 0:54], ones_row[:], scs(SC_H, 54),
                         start=True, stop=True)
        bc = pool.tile([P, 54], f32, tag="bc")
        V.tensor_copy(bc[:], pbc[:, 0:54])
        # bc cols: H (w,b,i,j) at w*18+b*9+i*3+j; h0 at 36+w*6+b*3+i;
        #          ac at 48+c; mcac at 51+c
        # ---- epilogue on this core's N-slice ---------------------------
        # tiles with free layout (b, c|i, f) = [128, 54]; walrus caps DVE
        # ops at 2 free dims, so loops go over (w, b[, i]).
        fcps = fcp_sl[:].rearrange("p (b c f) -> p b c f", b=B, c=C)
        mt = {}
        for w in range(2):
            prod = ep.tile([P, B * C * C * FS], f32, tag="prod")  # (b,i,j,f)
            pv = prod[:].rearrange("p (b i j f) -> p b i j f", b=B, i=C, j=C)
            for b in range(B):
                for i in range(C):
                    # bc H col block for (w,b,i): j contiguous
                    hcol = bc[:, w * 18 + b * 9 + i * 3:
                              w * 18 + b * 9 + i * 3 + 3]
                    V.tensor_mul(
                        pv[:, b, i],
                        fcps[:, b],
                        hcol.unsqueeze(2).broadcast_to((P, C, FS)))
            red = ep.tile([P, B * C * FS], f32, tag="red")
            rv = red[:].rearrange("p (b i f) -> p b i f", b=B, i=C)
            pj = prod[:].rearrange("p (b i j f) -> p b i f j", b=B, i=C, j=C)
            for b in range(B):
                for i in range(C):
                    V.reduce_sum(rv[:, b, i], pj[:, b, i],
                                 axis=mybir.AxisListType.X)
            res = ep.tile([P, B * C * FS], f32, tag=f"mt{w}")
            V.scalar_tensor_tensor(
                out=res[:].rearrange("p (g f) -> p g f", f=FS),
                in0=bc[:, 36 + w * 6:42 + w * 6]
                .unsqueeze(2).broadcast_to((P, 6, FS)),
                scalar=-1.0,
                in1=red[:].rearrange("p (g f) -> p g f", f=FS),
                op0=Alu.mult, op1=Alu.add)
            mt[w] = res

        msq = ep.tile([P, B * C * FS], f32, tag="msq")
        nc.scalar.activation(msq[:], mt[0][:], Act.Square)
        s2 = ep.tile([P, B * C * FS], f32, tag="s2")
        V.tensor_sub(s2[:], mt[1][:], msq[:])
        s2c = ep.tile([P, B * C * FS], f32, tag="s2c")
        V.tensor_scalar_max(s2c[:], s2[:], 0.0)
        st = ep.tile([P, B * C * FS], f32, tag="st")
        nc.scalar.activation(st[:], s2c[:], Act.Sqrt)

        fcsv = fc_sl[:].rearrange("p (b c f) -> p b c f", b=B, c=C)
        cnt = ep.tile([P, B * C * FS], f32, tag="cnt")
        cv_ = cnt[:].rearrange("p (b c f) -> p b c f", b=B, c=C)
        for b in range(B):
            V.tensor_mul(cv_[:, b], fcsv[:, b],
                         bc[:, 48:51].unsqueeze(2).broadcast_to((P, C, FS)))
            V.scalar_tensor_tensor(
                out=cv_[:, b],
                in0=bc[:, 51:54].unsqueeze(2).broadcast_to((P, C, FS)),
                scalar=-1.0, in1=cv_[:, b], op0=Alu.mult, op1=Alu.add)
        out_t = ep.tile([P, B * C * FS], f32, tag="outt")
        V.tensor_mul(out_t[:], st[:], cnt[:])
        V.tensor_add(out_t[:], out_t[:], mt[0][:])

        nc.sync.dma_start(
            out_sl.ap().rearrange("b c (p f) -> p b c f", p=P),
            out_t[:].rearrange("p (b c f) -> p b c f", b=B, c=C))


def build():
    nc = bacc.Bacc("TRN2", target_bir_lowering=False, debug=False,
                   num_devices=NCORES)
    dr = {}
    for name in FULL_INPUTS:
        dr[name] = nc.dram_tensor(name, [B, C, N], f32, kind="ExternalInput")
    dr["F_cp_sl"] = nc.dram_tensor("F_cp_sl", [B, C, NS], f32,
                                   kind="ExternalInput")
    dr["F_c_sl"] = nc.dram_tensor("F_c_sl", [B, C, NS], f32,
                                  kind="ExternalInput")
    out_sl = nc.dram_tensor("out_sl", [B, C, NS], f32, kind="ExternalOutput")
    with tile.TileContext(nc) as tc:
        _body(tc, dr, out_sl)
    nc.compile()
    return nc


_NC = None


def _get_nc():
    global _NC
    if _NC is None:
        _NC = build()
    return _NC


def make_in_maps(inputs):
    full = {k: np.ascontiguousarray(
        np.asarray(inputs[k], dtype=np.float32).reshape(B, C, N))
        for k in FULL_INPUTS}
    in_maps = []
    for r in range(NCORES):
        m = dict(full)
        sl = slice(r * NS, (r + 1) * NS)
        m["F_cp_sl"] = np.ascontiguousarray(full["F_c_previous"][:, :, sl])
        m["F_c_sl"] = np.ascontiguousarray(full["F_c"][:, :, sl])
        in_maps.append(m)
    return in_maps


def kernel(**inputs):
    nc = _get_nc()
    res = run_bass_kernel_spmd(nc, make_in_maps(inputs),
                               core_ids=list(range(NCORES)))
    return np.concatenate([res.results[r]["out_sl"] for r in range(NCORES)],
                          axis=2)


# revision 16
# speedup vs baseline: 1.2117x; 1.2117x over previous
"""AdaAttNStar fused kernel for 8 TRN2 NeuronCores.

Algebraic collapse: the reference builds A = Q^T K ([B, N, N]) explicitly, but
  M  = V A^T         = (V K^T) Q        ([B,C,C] Gram @ [B,C,N])
  S2 = V^2 A^T - M^2 = (V^2 K^T) Q - M^2
so the whole computation reduces to channel Grams ([B,3,3]), per-channel
normalization statistics, and one elementwise epilogue over [B,C,N].

Distribution: stats/Grams are global reductions, and on-chip collectives have a
multi-microsecond latency floor, so every core redundantly computes the (cheap)
reduction phase from the full inputs and epilogues only its 1/8 slice of N.
Per-core output slices are concatenated on the host.

Layouts: big tensors live in SBUF as [128, (b, c, f=72)] with spatial index
s = p*72 + f; reductions over s happen free-dim-first (DVE per partition) then
across partitions with a ones-column matmul on the PE. Input DMAs are spread
across the sync/scalar/tensor HWDGE queues so their ~0.8us issue costs overlap.
"""

import numpy as np

import concourse.bass as bass
import concourse.bacc as bacc
import concourse.tile as tile
from concourse import mybir
from concourse.bass_utils import run_bass_kernel_spmd

B, C, N = 2, 3, 9216
P, F = 128, 72            # N = P * F
NCORES = 8
NS, FS = N // NCORES, F // NCORES     # 1152, 9
MTOT = B * N              # 18432 elements per channel for the norm stats
EPS = 1e-12
f32 = mybir.dt.float32
Alu = mybir.AluOpType
Act = mybir.ActivationFunctionType
X = mybir.AxisListType.X

FULL_INPUTS = ["F_c", "F_s", "F_c_previous", "F_s_previous"]

# acc / sc column maps.  acc (per-partition partials, [128, 84]):
#   0:36  Gram accums, order (w,b,i,j)   w=0: V*Fsp, w=1: V^2*Fsp
#  36:54  sums  per (t,b,c), t in (fc, fcp, fsp)
#  54:60  F_s sums  (b,i)   (= sv)
#  60:66  F_s sumsq (b,i)   (= sv2)
#  66:84  sumsq per (t,b,c)
SC_R = 0
SC_SUM = 36
SC_SV = 54
SC_SV2 = 60
SC_SS = 66
SC_UP = 84      # b-pooled sums, t in {fc,fcp,fsp}          [9]
SC_SSP = 93     # b-pooled sumsq                            [9]
SC_Q = 102      # UP^2                                      [9]
SC_VARM = 111   # SSP - Q/MTOT  (= var*(MTOT-1))            [9]
SC_STD = 120    # sqrt(VARM/(MTOT-1))                       [9]
SC_SE = 129     # STD + eps                                 [9]
SC_A = 138      # 1/(STD+eps): ac=138, aq=141, ak=144       [9]
SC_GRID = 147   # sv_w[b,i] * UP[fsp, j]                    [36]
SC_RC = 183     # R - GRID/MTOT                             [36]
SC_AKQ = 219    # ak*aq                                     [3]
SC_H = 222      # RC*akq_j     <- broadcast region start    [36]
SC_H0 = 258     # sum_j H*mq_j                              [12]
SC_AC2 = 270    # ac replicated per b                       [6]
SC_MCAC2 = 276  # mc*ac replicated per b                    [6]
SC_HM = 288     # scratch H*UP[fcp,j]/MTOT                  [36]
NBC = 60        # broadcast region size (SC_H .. SC_H+60)


def _body(tc, dr, out_sl):
    nc = tc.nc
    V = nc.vector
    with (
        tc.tile_pool(name="main", bufs=1) as pool,
        tc.tile_pool(name="scr", bufs=4) as scr,
        tc.tile_pool(name="ep", bufs=2) as ep,
        tc.tile_pool(name="ps", bufs=1, space="PSUM") as pp,
    ):
        # ---- input DMAs, spread across HWDGE queues ---------------------
        def load_full(name, key, eng):
            tl = pool.tile([P, B * C * F], f32, tag=key)
            eng.dma_start(
                tl[:].rearrange("p (b c f) -> p b c f", b=B, c=C),
                dr[name].ap().rearrange("b c (p f) -> p b c f", p=P))
            return tl

        t = {}
        t["fs"] = load_full("F_s", "fs", nc.sync)
        t["fsp"] = load_full("F_s_previous", "fsp", nc.scalar)
        t["fcp"] = load_full("F_c_previous", "fcp", nc.sync)
        t["fc"] = load_full("F_c", "fc", nc.sync)

        fcp_sl = pool.tile([P, B * C * FS], f32, tag="fcpsl")
        nc.sync.dma_start(
            fcp_sl[:].rearrange("p (b c f) -> p b c f", b=B, c=C),
            dr["F_cp_sl"].ap().rearrange("b c (p f) -> p b c f", p=P))
        fc_sl = pool.tile([P, B * C * FS], f32, tag="fcsl")
        nc.sync.dma_start(
            fc_sl[:].rearrange("p (b c f) -> p b c f", b=B, c=C),
            dr["F_c_sl"].ap().rearrange("b c (p f) -> p b c f", p=P))

        # F_cp slice replicated over i -> (b, i, j, f), built by gpsimd
        fcp3 = pool.tile([P, B * C * C * FS], f32, tag="fcp3")
        f3v = fcp3[:].rearrange("p (b i j f) -> p b i j f", b=B, i=C, j=C)
        fslv = fcp_sl[:].rearrange("p (b j f) -> p b j f", b=B, j=C)
        for b in range(B):
            for i in range(C):
                nc.gpsimd.tensor_copy(f3v[:, b, i], fslv[:, b])

        def v4(tl, f=F):   # [128, (b c f)] -> [128, b, c, f]
            return tl[:].rearrange("p (b c f) -> p b c f", b=B, c=C, f=f)

        # ---- squares (ACT) ---------------------------------------------
        v2 = pool.tile([P, B * C * F], f32, tag="v2")
        nc.scalar.activation(v2[:], t["fs"][:], Act.Square)
        sq = {"fs": v2}
        for key in ["fc", "fcp", "fsp"]:
            s = pool.tile([P, B * C * F], f32, tag=f"sq{key}")
            nc.scalar.activation(s[:], t[key][:], Act.Square)
            sq[key] = s

        ones_col = pool.tile([P, 1], f32, tag="onesc")
        nc.gpsimd.memset(ones_col[:], 1.0)
        ones_row = pool.tile([1, P], f32, tag="onesr")
        nc.gpsimd.memset(ones_row[:], 1.0)

        # ---- per-partition partial reductions (DVE) --------------------
        acc = pool.tile([P, 84], f32, tag="acc")
        fsv, v2v, fspv = v4(t["fs"]), v4(v2), v4(t["fsp"])
        for w, src in enumerate([fsv, v2v]):
            for b in range(B):
                for i in range(C):
                    for j in range(C):
                        q = ((w * B + b) * C + i) * C + j
                        wscr = scr.tile([P, F], f32, tag="wscr")
                        V.scalar_tensor_tensor(
                            out=wscr[:],
                            in0=src[:, b, i, :], scalar=1.0,
                            in1=fspv[:, b, j, :],
                            op0=Alu.mult, op1=Alu.mult,
                            accum_out=acc[:, q:q + 1])
        for ti, key in enumerate(["fc", "fcp", "fsp"]):
            V.reduce_sum(
                acc[:, 36 + ti * 6:42 + ti * 6].rearrange(
                    "p (b c) -> p b c", b=B), v4(t[key]), axis=X)
            V.reduce_sum(
                acc[:, 66 + ti * 6:72 + ti * 6].rearrange(
                    "p (b c) -> p b c", b=B), v4(sq[key]), axis=X)
        V.reduce_sum(acc[:, 54:60].rearrange("p (b c) -> p b c", b=B),
                     v4(t["fs"]), axis=X)
        V.reduce_sum(acc[:, 60:66].rearrange("p (b c) -> p b c", b=B),
                     v4(v2), axis=X)

        # ---- cross-partition reduction on the PE -----------------------
        p1 = pp.tile([1, 96], f32, tag="p1")
        nc.tensor.matmul(p1[:, 0:84], ones_col[:], acc[:, 0:84],
                         start=True, stop=True)

        # ---- scalar phase on [1, *] ------------------------------------
        sc = pool.tile([1, 384], f32, tag="sc")

        def scs(base, n):
            return sc[:, base:base + n]

        V.tensor_copy(scs(0, 84), p1[:, 0:84])
        sum_t = scs(SC_SUM, 18).rearrange("o (t b c) -> o t b c", t=3, b=B)
        ss_t = scs(SC_SS, 18).rearrange("o (t b c) -> o t b c", t=3, b=B)
        V.tensor_add(scs(SC_UP, 9), sum_t[:, :, 0, :], sum_t[:, :, 1, :])
        V.tensor_add(scs(SC_SSP, 9), ss_t[:, :, 0, :], ss_t[:, :, 1, :])
        V.tensor_mul(scs(SC_Q, 9), scs(SC_UP, 9), scs(SC_UP, 9))
        V.scalar_tensor_tensor(
            out=scs(SC_VARM, 9), in0=scs(SC_Q, 9), scalar=-1.0 / MTOT,
            in1=scs(SC_SSP, 9), op0=Alu.mult, op1=Alu.add)
        nc.scalar.activation(scs(SC_STD, 9), scs(SC_VARM, 9), Act.Sqrt,
                             scale=1.0 / (MTOT - 1))
        V.tensor_scalar_add(scs(SC_SE, 9), scs(SC_STD, 9), EPS)
        V.reciprocal(scs(SC_A, 9), scs(SC_SE, 9))

        # grid[w,b,i,j] = sv_w[b,i] * UP[fsp, j]; vw = sc[54:66] contiguous
        up_fsp = scs(SC_UP + 6, 3)
        V.tensor_mul(
            scs(SC_GRID, 36).rearrange("o (g j) -> o g j", j=C),
            scs(SC_SV, 12).unsqueeze(2).broadcast_to((1, 12, 3)),
            up_fsp.unsqueeze(1).broadcast_to((1, 12, 3)))
        V.scalar_tensor_tensor(
            out=scs(SC_RC, 36), in0=scs(SC_GRID, 36), scalar=-1.0 / MTOT,
            in1=scs(SC_R, 36), op0=Alu.mult, op1=Alu.add)
        V.tensor_mul(scs(SC_AKQ, 3), scs(SC_A + 6, 3), scs(SC_A + 3, 3))
        V.tensor_mul(
            scs(SC_H, 36).rearrange("o (g j) -> o g j", j=C),
            scs(SC_RC, 36).rearrange("o (g j) -> o g j", j=C),
            scs(SC_AKQ, 3).unsqueeze(1).broadcast_to((1, 12, 3)))
        V.scalar_tensor_tensor(
            out=scs(SC_HM, 36).rearrange("o (g j) -> o g j", j=C),
            in0=scs(SC_H, 36).rearrange("o (g j) -> o g j", j=C),
            scalar=1.0 / MTOT,
            in1=scs(SC_UP + 3, 3).unsqueeze(1).broadcast_to((1, 12, 3)),
            op0=Alu.mult, op1=Alu.mult)
        V.reduce_sum(scs(SC_H0, 12),
                     scs(SC_HM, 36).rearrange("o (g j) -> o g j", j=C), axis=X)
        V.tensor_copy(scs(SC_AC2, 6),
                      scs(SC_A, 3).unsqueeze(1).broadcast_to((1, 2, 3)))
        V.scalar_tensor_tensor(
            out=scs(SC_MCAC2, 6).rearrange("o (b c) -> o b c", b=B),
            in0=scs(SC_UP, 3).unsqueeze(1).broadcast_to((1, 2, 3)),
            scalar=1.0 / MTOT,
            in1=scs(SC_AC2, 6).rearrange("o (b c) -> o b c", b=B),
            op0=Alu.mult, op1=Alu.mult)

        # ---- broadcast the 60 epilogue scalars to all partitions -------
        pbc = pp.tile([P, 64], f32, tag="pbc")
        nc.tensor.matmul(pbc[:, 0:NBC], ones_row[:], scs(SC_H, NBC),
                         start=True, stop=True)
        bc = pool.tile([P, NBC], f32, tag="bc")
        V.tensor_copy(bc[:], pbc[:, 0:NBC])
        # bc cols: H (w,b,i,j) at w*18+b*9+i*3+j; h0 at 36+(w,b,i);
        #          ac2 at 48+(b,c); mcac2 at 54+(b,c)

        # ---- epilogue on this core's N-slice ---------------------------
        mt = {}
        for w in range(2):
            prod = ep.tile([P, B * C * C * FS], f32, tag="prod")
            V.tensor_mul(
                prod[:].rearrange("p (g f) -> p g f", f=FS),
                fcp3[:].rearrange("p (g f) -> p g f", f=FS),
                bc[:, w * 18:(w + 1) * 18].unsqueeze(2)
                .broadcast_to((P, 18, FS)))
            red = ep.tile([P, B * C * FS], f32, tag="red")
            V.reduce_sum(
                red[:].rearrange("p (g f) -> p g f", f=FS),
                prod[:].rearrange("p (g j f) -> p g f j", j=C, f=FS), axis=X)
            res = ep.tile([P, B * C * FS], f32, tag=f"mt{w}")
            V.scalar_tensor_tensor(
                out=res[:].rearrange("p (g f) -> p g f", f=FS),
                in0=bc[:, 36 + w * 6:42 + w * 6]
                .unsqueeze(2).broadcast_to((P, 6, FS)),
                scalar=-1.0,
                in1=red[:].rearrange("p (g f) -> p g f", f=FS),
                op0=Alu.mult, op1=Alu.add)
            mt[w] = res

        msq = ep.tile([P, B * C * FS], f32, tag="msq")
        nc.scalar.activation(msq[:], mt[0][:], Act.Square)
        s2 = ep.tile([P, B * C * FS], f32, tag="s2")
        V.tensor_sub(s2[:], mt[1][:], msq[:])
        s2c = ep.tile([P, B * C * FS], f32, tag="s2c")
        V.tensor_scalar_max(s2c[:], s2[:], 0.0)
        st = ep.tile([P, B * C * FS], f32, tag="st")
        nc.scalar.activation(st[:], s2c[:], Act.Sqrt)

        cnt = ep.tile([P, B * C * FS], f32, tag="cnt")
        V.tensor_mul(
            cnt[:].rearrange("p (g f) -> p g f", f=FS),
            fc_sl[:].rearrange("p (g f) -> p g f", f=FS),
            bc[:, 48:54].unsqueeze(2).broadcast_to((P, 6, FS)))
        V.scalar_tensor_tensor(
            out=cnt[:].rearrange("p (g f) -> p g f", f=FS),
            in0=bc[:, 54:60].unsqueeze(2).broadcast_to((P, 6, FS)),
            scalar=-1.0,
            in1=cnt[:].rearrange("p (g f) -> p g f", f=FS),
            op0=Alu.mult, op1=Alu.add)
        out_t = ep.tile([P, B * C * FS], f32, tag="outt")
        V.tensor_mul(out_t[:], st[:], cnt[:])
        V.tensor_add(out_t[:], out_t[:], mt[0][:])

        nc.sync.dma_start(
            out_sl.ap().rearrange("b c (p f) -> p b c f", p=P),
            out_t[:].rearrange("p (b c f) -> p b c f", b=B, c=C))


def build():
    nc = bacc.Bacc("TRN2", target_bir_lowering=False, debug=False,
                   num_devices=NCORES)
    dr = {}
    for name in FULL_INPUTS:
        dr[name] = nc.dram_tensor(name, [B, C, N], f32, kind="ExternalInput")
    dr["F_cp_sl"] = nc.dram_tensor("F_cp_sl", [B, C, NS], f32,
                                   kind="ExternalInput")
    dr["F_c_sl"] = nc.dram_tensor("F_c_sl", [B, C, NS], f32,
                                  kind="ExternalInput")
    out_sl = nc.dram_tensor("out_sl", [B, C, NS], f32, kind="ExternalOutput")
    with tile.TileContext(nc) as tc:
        _body(tc, dr, out_sl)
    nc.compile()
    return nc


_NC = None


def _get_nc():
    global _NC
    if _NC is None:
        _NC = build()
    return _NC


def make_in_maps(inputs):
    full = {k: np.ascontiguousarray(
        np.asarray(inputs[k], dtype=np.float32).reshape(B, C, N))
        for k in FULL_INPUTS}
    in_maps = []
    for r in range(NCORES):
        m = dict(full)
        sl = slice(r * NS, (r + 1) * NS)
        m["F_cp_sl"] = np.ascontiguousarray(full["F_c_previous"][:, :, sl])
        m["F_c_sl"] = np.ascontiguousarray(full["F_c"][:, :, sl])
        in_maps.append(m)
    return in_maps


def kernel(**inputs):
    nc = _get_nc()
    res = run_bass_kernel_spmd(nc, make_in_maps(inputs),
                               core_ids=list(range(NCORES)))
    return np.concatenate([res.results[r]["out_sl"] for r in range(NCORES)],
                          axis=2)
